# revision 14
# baseline (speedup 1.0000x reference)
"""Trainium2 Bass kernel for nn_CustomModel_52338471469275 (dense MLP).

Computes out = relu(input @ (S*THETA)^T + bias) @ weight + bias2
  input  [2048, 8192] f32
  S,THETA[1024, 8192] f32   (fused on host into W1 = S*THETA)
  weight [1024, 1024] f32
  out    [2048, 1024] f32

Sharding over 8 NeuronCores: 4 batch groups (512 rows each) x 2 hidden
halves (512 of the 1024 hidden units each).  Core (i, j) computes

  fT_ij  = relu(W1[jblk] @ x[iblk]^T + bias[jblk])          # [512, 512]
  outT_p = weight[jblk]^T @ fT_ij                           # [1024, 512]

i.e. a partial (contraction-split) second GEMM.  The host sums the two
j-partials per batch group, transposes, and adds bias2.  No on-device
collectives needed.

GEMM1's x and W1 operands are interleaved on the host into ONE fused
"pair" stream (per k-tile: the x lines and the w lines side by side),
and pair-blocks alternate between the two hardware DGE queues (SP and
ACT).  Rationale, measured on hardware: each queue tops out at ~205
GB/s and the Tile framework's 8-semaphore rotation caps one queue at
as few as 3 transfers in flight -- with separate x/w streams the w
queue starved the PE (4.7us of stalls, and one >2us gap re-throttled
the HAM clock gate).  The fused stream gives every k-tile both
operands with one semaphore and lets BOTH queues feed the single
stream: 410 GB/s aggregate against the warm PE's 300 GB/s demand.

All matmul operands are cast to bf16 on the host (fp32 PSUM
accumulation on device).  Measured end-to-end relative error vs the
fp32 reference is ~3e-3 (absmax-relative), from bf16 rounding.
"""

import os
import sys

import numpy as np

if "/opt/trn_rl_repo" not in sys.path:
    sys.path.insert(0, "/opt/trn_rl_repo")

import ml_dtypes

import concourse.bass as bass
import concourse.tile as tile
from concourse import mybir
from concourse._compat import checkenv
from concourse.bass_utils import run_bass_kernel_spmd

# The image's antenv stub lacks axon_hooks; if BASS_TRACE is set in the
# environment, run_bass_kernel_spmd imports it unconditionally. Provide a
# no-op fallback (trace is skipped, compile+run still work) unless a real
# hook module is already installed.
try:
    import antenv.axon_hooks  # noqa: F401
except ImportError:
    import types

    import antenv

    _hooks = types.ModuleType("antenv.axon_hooks")
    _hooks._hook = None
    _hooks.set_axon_ntff_profile_hook = lambda h: setattr(_hooks, "_hook", h)
    _hooks.get_axon_ntff_profile_hook = lambda: _hooks._hook
    sys.modules["antenv.axon_hooks"] = _hooks
    antenv.axon_hooks = _hooks

B, O, I = 2048, 1024, 8192
R, C = 4, 2                 # batch groups x hidden halves
BS, OS = B // R, O // C     # 512, 512
P = 128
N = BS                      # moving free dim per matmul
KT1 = I // P                # 64 k-tiles, GEMM1
MT1 = OS // P               # 4 m-tiles, GEMM1
KT2 = OS // P               # 4 k-tiles, GEMM2
MT2 = O // P                # 8 m-tiles, GEMM2

BF16 = mybir.dt.bfloat16
F32 = mybir.dt.float32

# k-tiles per fused pair-slab.  One pair-slab carries x AND w lines for
# its k-tiles (512 KB per k-tile pair... 256KB: 1KB x-line + 1KB w-line
# per partition per k-tile).  Small first blocks minimize latency to the
# first matmul; 2-tile blocks (4KB DMA lines) already reach the ~205
# GB/s per-queue ceiling; 4-tile blocks amortize trigger overhead in
# steady state.  Blocks alternate SP/ACT queues, so the fused stream is
# fed at 2x the per-queue rate.
SCHED = [1, 1] + [2] * 8 + [3, 3] + [4] * 10
assert sum(SCHED) == KT1
QKMAX = max(SCHED)

# PE warm-up matmuls issued before the first pair-slab lands: ~5x427ns
# cold N=512 + 4 fine-grained N=128 tail fillers.  This keeps the PE
# busy from ~8.0us (DVE memset + body entry) so the HAM clock gate
# (3.4us sustained-busy window) opens close to when real data arrives,
# and the warmup ends right as the first slab semaphore fires (~10.7us).
WARM512, WARM128 = 5, 4


def _pack_pairs(xT, wT):
    """Interleave xT and wT ([8192, 512] each) into the fused pair stream:
    for each SCHED block of QK k-tiles, a C-order [P, QK, 2, N] array
    whose per-partition line is (QK * 2 * N) contiguous elements --
    x line and w line side by side per k-tile."""
    out = np.empty((2 * I, N), dtype=xT.dtype)
    kt0 = 0
    row = 0
    for QK in SCHED:
        xb = xT[kt0 * P : (kt0 + QK) * P].reshape(QK, P, N)
        wb = wT[kt0 * P : (kt0 + QK) * P].reshape(QK, P, N)
        blk = np.stack([xb, wb], axis=2)          # [QK, P, 2, N]
        blk = blk.transpose(1, 0, 2, 3)           # [P, QK, 2, N]
        nrows = QK * 2 * P
        out[row : row + nrows] = blk.reshape(nrows, N)
        row += nrows
        kt0 += QK
    return out

_CACHE = {}
LAST_RESULTS = None  # BassKernelResults of the most recent run (for test.py)


def _split_multi_waits(nc, max_waits=1):
    """This container's walrus codegen rejects instructions carrying more
    than one semaphore wait ("Too many sync wait commands", CoreV3GenImpl).
    Tile's kernel-tail drain aggregates several; hoist the extras onto
    preceding same-engine NoOps (identical semantics: engines execute their
    stream in order)."""
    for fn in nc.m.functions:
        for blk in fn.blocks:
            new_insts = []
            for inst in blk.instructions:
                si = inst.sync_info
                waits = list(si.on_wait) if si and si.on_wait else []
                if len(waits) > max_waits:
                    extra, keep = waits[:-max_waits], waits[-max_waits:]
                    for k, w in enumerate(extra):
                        new_insts.append(
                            mybir.InstNoOp(
                                name=f"{inst.name}_wsplit{k}",
                                engine=inst.engine,
                                ins=[],
                                outs=[],
                                sync_info=mybir.SyncInfo(on_wait=[w], on_update=[]),
                            )
                        )
                    inst.sync_info = mybir.SyncInfo(
                        on_wait=keep,
                        on_update=list(si.on_update) if si.on_update else [],
                    )
                new_insts.append(inst)
            blk.instructions = new_insts


def _build_nc() -> bass.Bass:
    nc = bass.Bass()
    xw = nc.declare_dram_parameter("xw", [2 * I, N], BF16, isOutput=False)
    b1 = nc.declare_dram_parameter("b1", [P, MT1], F32, isOutput=False)
    w2 = nc.declare_dram_parameter("w2", [OS, O], BF16, isOutput=False)
    # bf16 partials: halves the output write traffic in the serial tail;
    # the host reduces the two j-partials in fp32
    outT = nc.declare_dram_parameter("outT", [O, BS], BF16, isOutput=True)

    with tile.TileContext(nc) as tc:
        with (
            tc.tile_pool(name="const", bufs=1) as const,
            tc.tile_pool(name="xw", bufs=14) as xwpool,
            tc.tile_pool(name="fp", bufs=1) as fpool,
            tc.tile_pool(name="op", bufs=8) as opool,
            tc.tile_pool(name="ps1", bufs=1, space="PSUM") as ps1,
            tc.tile_pool(name="ps2", bufs=4, space="PSUM") as ps2,
        ):
            # PE warm-up: ~2.6us of dummy matmuls while the first slabs
            # are in flight, so the HAM clock gate (1.2 -> 2.4 GHz, after
            # ~3.4us of sustained PE activity) opens around the time the
            # real accumulation starts.  Keep warmup PE-only.
            warm = const.tile([P, N], BF16)
            nc.vector.memset(warm[:], 0.0)
            scratch = const.tile([P, 1], BF16)
            wps = ps2.tile([P, N], F32, tag="p2g")
            for _ in range(WARM512):
                nc.tensor.matmul(wps[:], warm[:, :P], warm[:],
                                 start=True, stop=True)
            for _ in range(WARM128):
                nc.tensor.matmul(wps[:, :P], warm[:, :P], warm[:, :P],
                                 start=True, stop=True)

            # GEMM1: logitsT[m1blk, :] += W1T[ktblk, m1blk]^T @ xT[ktblk, :]
            # One PSUM bank tile per m1 so the bias+relu for m1=0 can start
            # as soon as its own final matmul retires (not all four).
            pst = [ps1.tile([P, N], F32, tag=f"ps1_{m1}", name=f"pst{m1}")
                   for m1 in range(MT1)]
            kt0 = 0
            row = 0
            for kb, QK in enumerate(SCHED):
                xwt = xwpool.tile([P, QKMAX, 2, N], BF16, tag="xw")
                src = xw[row : row + QK * 2 * P, :].rearrange(
                    "(p q two) n -> p q two n", p=P, two=2
                )
                if kb % 2 == 0:
                    nc.sync.dma_start(xwt[:, :QK, :, :], src)
                else:
                    nc.scalar.dma_start(xwt[:, :QK, :, :], src)
                if kb == 11:
                    # Dummy activation AFTER the latency-critical ramp
                    # triggers: walrus hoists the ~1.3us ACT_TABLE_LOAD in
                    # front of the first ACTIVATE in program order, and if
                    # the dummy sits early it lands BETWEEN ACT's ramp
                    # triggers and delays the Q10 pairs by ~1us (measured).
                    # Here the load runs in ACT's slack, long before the
                    # first real relu needs the table.
                    nc.scalar.activation(
                        scratch[:], warm[:, :1],
                        mybir.ActivationFunctionType.Relu,
                    )
                last_block = kb == len(SCHED) - 1
                if not last_block:
                    for q in range(QK):
                        for m1 in range(MT1):
                            nc.tensor.matmul(
                                pst[m1][:],
                                xwt[:, q, 1, m1 * P : (m1 + 1) * P],
                                xwt[:, q, 0, :],
                                start=(kt0 + q == 0),
                                stop=False,
                            )
                else:
                    # Final block runs m1-OUTER so pst[0..2] finish (and
                    # their bias+relu casts run) under the tail of GEMM1's
                    # matmul stream instead of serializing after it.
                    for m1 in range(MT1):
                        for q in range(QK):
                            nc.tensor.matmul(
                                pst[m1][:],
                                xwt[:, q, 1, m1 * P : (m1 + 1) * P],
                                xwt[:, q, 0, :],
                                start=False,
                                stop=(q == QK - 1),
                            )
                row += QK * 2 * P
                kt0 += QK

            # constants for the second GEMM (SP has slack between slab
            # triggers; avoiding gpsimd skips its costly SWDGE drain)
            b1_t = const.tile([P, MT1], F32)
            nc.sync.dma_start(b1_t[:], b1[:])
            # w2 host-packed p-major: one contiguous 8 KB line per partition
            w2_sb = const.tile([P, KT2, O], BF16)
            nc.sync.dma_start(
                w2_sb[:], w2[:].rearrange("(k p) o -> p k o", p=P)
            )

            # bias + relu, cast to bf16.  ft0/ft2 on ACT, ft1 on DVE --
            # these three complete while GEMM1's final m1=3 matmuls still
            # run (m1-outer final block).  ft3 is the only relu on the
            # critical path, so it is split across DVE+ACT halves (~0.4us
            # instead of 0.75) and GEMM2's k=3 matmul barely waits.
            ft = [fpool.tile([P, N], BF16, tag=f"f_{m1}", name=f"ft{m1}")
                  for m1 in range(MT1)]
            h = N // 2
            for m1 in range(MT1):
                if m1 == MT1 - 1:
                    nc.vector.tensor_scalar(
                        ft[m1][:, :h],
                        pst[m1][:, :h],
                        b1_t[:, m1 : m1 + 1],
                        0.0,
                        mybir.AluOpType.add,
                        mybir.AluOpType.max,
                    )
                    nc.scalar.activation(
                        ft[m1][:, h:],
                        pst[m1][:, h:],
                        mybir.ActivationFunctionType.Relu,
                        bias=b1_t[:, m1 : m1 + 1],
                    )
                elif m1 % 2 == 0:
                    nc.scalar.activation(
                        ft[m1][:],
                        pst[m1][:],
                        mybir.ActivationFunctionType.Relu,
                        bias=b1_t[:, m1 : m1 + 1],
                    )
                else:
                    nc.vector.tensor_scalar(
                        ft[m1][:],
                        pst[m1][:],
                        b1_t[:, m1 : m1 + 1],
                        0.0,
                        mybir.AluOpType.add,
                        mybir.AluOpType.max,
                    )

            # GEMM2 (partial over this core's hidden half):
            # outT[m2blk, :] = sum_kt2 w2[kt2blk, m2blk]^T @ fT[kt2blk, :]
            # one m2 per PSUM bank, 4 banks in flight; PSUM->SBUF casts
            # alternate DVE/ACT and output DMAs alternate SP/ACT queues so
            # the post-GEMM2 drain chain is short.
            for m2 in range(MT2):
                ot = opool.tile([P, N], BF16)
                if m2 == MT2 - 1:
                    # final block: its own recycled GEMM1 bank, computed as
                    # TWO column-chunks with separate accumulation groups,
                    # so chunk A's cast+DMA overlap chunk B's matmuls and
                    # the post-last-matmul drain is only one half-width
                    # cast + trigger + 64 KB flight.  Both chunk casts ride
                    # DVE (measured: ACT's last cast slips ~0.8us behind
                    # its dependences); triggers split Sync/Scalar queues.
                    # Each chunk gets its OWN recycled GEMM1 bank: sharing
                    # one tile makes chunk B's matmuls wait on chunk A's
                    # cast (tile-granular WAR), a measured 0.5us PE gap.
                    p2c = [
                        ps1.tile([P, h], F32, tag="ps1_1", name="p2ra"),
                        ps1.tile([P, h], F32, tag="ps1_2", name="p2rb"),
                    ]
                    for ci, (c0, c1) in enumerate([(0, h), (h, N)]):
                        p2 = p2c[ci]
                        for kt in range(KT2):
                            nc.tensor.matmul(
                                p2[:],
                                w2_sb[:, kt, m2 * P : (m2 + 1) * P],
                                ft[kt][:, c0:c1],
                                start=(kt == 0),
                                stop=(kt == KT2 - 1),
                            )
                        nc.vector.tensor_copy(ot[:, c0:c1], p2[:])
                        if ci == 0:
                            nc.sync.dma_start(
                                outT[m2 * P : (m2 + 1) * P, c0:c1],
                                ot[:, c0:c1],
                            )
                        else:
                            nc.scalar.dma_start(
                                outT[m2 * P : (m2 + 1) * P, c0:c1],
                                ot[:, c0:c1],
                            )
                    continue
                if m2 < MT2 - 2:
                    p2 = ps2.tile([P, N], F32, tag="p2g", name=f"p2_{m2}")
                else:
                    # GEMM1's accumulator banks are free once the relus have
                    # read them; reusing one widens the bank rotation so
                    # casts never gate the matmuls
                    p2 = ps1.tile([P, N], F32, tag="ps1_0", name=f"p2r_{m2}")
                for kt in range(KT2):
                    nc.tensor.matmul(
                        p2[:],
                        w2_sb[:, kt, m2 * P : (m2 + 1) * P],
                        ft[kt][:],
                        start=(kt == 0),
                        stop=(kt == KT2 - 1),
                    )
                # halve every PSUM->SBUF cast across DVE+ACT: the PSUM bank
                # frees in ~345ns instead of ~690, so the 4-bank rotation
                # never paces GEMM2.  Output DMAs alternate SP/ACT queues
                # (both have slack in the output phase).
                nc.vector.tensor_copy(ot[:, :h], p2[:, :h])
                nc.scalar.activation(
                    ot[:, h:], p2[:, h:], mybir.ActivationFunctionType.Copy
                )
                if m2 % 2 == 0:
                    nc.sync.dma_start(outT[m2 * P : (m2 + 1) * P, :], ot[:])
                else:
                    nc.scalar.dma_start(outT[m2 * P : (m2 + 1) * P, :], ot[:])

    _split_multi_waits(nc)
    return nc


def kernel(input, S, THETA, bias, weight, bias2):
    global LAST_RESULTS
    if "nc" not in _CACHE:
        _CACHE["nc"] = _build_nc()
    nc = _CACHE["nc"]

    bf16 = ml_dtypes.bfloat16
    input = np.asarray(input, dtype=np.float32)
    W1 = np.asarray(S, dtype=np.float32) * np.asarray(THETA, dtype=np.float32)
    bias = np.asarray(bias, dtype=np.float32)
    weight = np.asarray(weight, dtype=np.float32)
    bias2 = np.asarray(bias2, dtype=np.float32)

    xT_g = [
        np.ascontiguousarray(input[i * BS : (i + 1) * BS, :].T).astype(bf16)
        for i in range(R)
    ]
    w1T_g = [
        np.ascontiguousarray(W1[j * OS : (j + 1) * OS, :].T).astype(bf16)
        for j in range(C)
    ]
    xw_g = {}
    for i in range(R):
        for j in range(C):
            xw_g[(i, j)] = _pack_pairs(xT_g[i], w1T_g[j])
    b1_g = [
        np.ascontiguousarray(bias[j * OS : (j + 1) * OS].reshape(MT1, P).T)
        for j in range(C)
    ]
    w2_g = [weight[j * OS : (j + 1) * OS, :].astype(bf16) for j in range(C)]

    in_maps = []
    for i in range(R):
        for j in range(C):
            in_maps.append(
                {"xw": xw_g[(i, j)], "b1": b1_g[j], "w2": w2_g[j]}
            )

    res = run_bass_kernel_spmd(
        nc,
        in_maps,
        core_ids=list(range(R * C)),
        trace=checkenv("BASS_TRACE"),
    )
    LAST_RESULTS = res

    out = np.empty((B, O), dtype=np.float32)
    for i in range(R):
        acc = res.results[i * C]["outT"].astype(np.float32)
        for j in range(1, C):
            acc = acc + res.results[i * C + j]["outT"]
        out[i * BS : (i + 1) * BS, :] = acc.T
    out += bias2[None, :]
    return out
